# revision 14
# baseline (speedup 1.0000x reference)
"""Trainium2 Bass kernel for nn_CrfRnnLayerSPAT (segment_reduce).

Strategy
--------
Host: sort pixels by superpixel id, pack each segment's pixels into whole
"ranges" of K slots (padded), bin-pack whole segments onto 8 cores x NCHUNK
chunks (128 ranges per chunk, one range per SBUF partition).  Every
device-side op then becomes dense.  xs/out are laid out [ch, r, c, k] so
each DMA is contiguous per partition row.

  phase 1:  et = exp(x)                        (ACT, persists in SBUF)
            s = sum_c et                       (DVE + Pool partial chains)
            lsum = ln2/128*(sum_k bits(s)-K*B) (DVE reduce on the bf16 bit
                                                pattern: Schraudolph ln)
            pacc[ls,c,:] = OH^T @ x            (PE, accumulated over k-blocks)
  tiny:     B[ls,c] = sum_k pacc - (LS - corr)     (DVE + tiny PE matmul)
            Bex[r,c] = OHT^T @ B                   (PE)
            w = exp(B)((lw0-hw0) + (lw1-hw1)exp(498B))   (ACT + DVE, small)
  phase 2:  bits(f) = bits(s)+B0 - bits(et_c)  (DVE uint16 subtract: log-
                                                domain division f = s/et,
                                                no second exp pass over x)
            og = f*w_c + a                     (DVE / ACT-Identity, f16)

The reference's log(q+eps) is computed as x - ln(s) (eps dropped) and the
phase-2 exp(logsum - logq) via bit-pattern arithmetic: both approximations
only perturb quantities that the downstream exp() underflows to exactly 0
in fp32 anyway (B ~ -6300), so the final output is unaffected.
f_att = exp(499B - logq) = f_sp * exp(498B), folded into the per-range w.

No collectives: segments are whole per (core, chunk).  Output is written in
sorted order (f16) and un-permuted on the host.
"""

import os

os.environ.setdefault("MYCRO_LOCAL_CACHE", "1")

import numpy as np
import ml_dtypes

C = 21
H = W = 1024
NPIX = H * W
NSEG = 500
NCORES = 8
NCHUNK = 2
RPC = 128                  # ranges per chunk (one per SBUF partition)
K = 1120 // NCHUNK         # slots per range
S_CH = RPC * K             # slots per chunk
S = NCHUNK * S_CH          # slots per core = 143360
LSEG = {1: 72, 2: 40, 4: 24}[NCHUNK]  # local segment table width (last = dummy)
NDMA = 5                   # class-slices per chunk xs/og DMA
IN_DT = "bf16"             # "bf16" or "f8" (fp8e4m3 input, half the read traffic)
KB = 20                    # k-block for the pacc matmul (C*KB*4B <= 2KB PSUM bank)
CSUM_POOL = 5              # classes of the s-sum accumulated on Pool (of 21)
TS_ACT = 2                 # phase-2 og ops done on ACT Identity per chunk (of 21)

B0 = 16256                 # bf16 bit pattern of 1.0 (0x3F80)
LN2_128 = float(np.log(2.0) / 128.0)   # ln per bf16 ulp-of-exponent

_BF16 = ml_dtypes.bfloat16
_F8 = ml_dtypes.float8_e4m3fn

_cache = {}


def _build_nc(reps=1):
    import concourse.bacc as bacc
    import concourse.mybir as mybir
    from concourse.tile import TileContext

    f32 = mybir.dt.float32
    bf16 = mybir.dt.bfloat16
    f16 = mybir.dt.float16
    u16 = mybir.dt.uint16
    xdt = mybir.dt.bfloat16 if IN_DT == "bf16" else mybir.dt.float8e4
    AF = mybir.ActivationFunctionType
    OP = mybir.AluOpType
    AX = mybir.AxisListType

    nc = bacc.Bacc()

    xs_d = nc.dram_tensor("xs", [NCHUNK, RPC, C, K], xdt, kind="ExternalInput")
    oh_d = nc.dram_tensor("oh", [NCHUNK, RPC, LSEG], f32, kind="ExternalInput")
    oht_d = nc.dram_tensor("oht", [NCHUNK, LSEG, RPC], f32, kind="ExternalInput")
    corrt_d = nc.dram_tensor("corrt", [NCHUNK, LSEG, 1], f32, kind="ExternalInput")
    spb_d = nc.dram_tensor("spb", [RPC, C], f32, kind="ExternalInput")
    tpb_d = nc.dram_tensor("tpb", [RPC, C], f32, kind="ExternalInput")
    asc_d = nc.dram_tensor("asc", [RPC, 1], f32, kind="ExternalInput")
    out_d = nc.dram_tensor("out", [NCHUNK, RPC, C, K], f16, kind="ExternalOutput")

    # class-slice boundaries for the xs/og DMAs
    csl = [(C * i) // NDMA for i in range(NDMA + 1)]
    nkb = K // KB

    with TileContext(nc) as tc:
        with (
            tc.tile_pool(name="persist", bufs=1) as pp,
            tc.tile_pool(name="stream", bufs=3) as sp,
            tc.tile_pool(name="ostream", bufs=2) as op_pool,
            tc.tile_pool(name="psum", bufs=1, space="PSUM") as qp,
        ):
          for _rep in range(reps):
            # ---- shared small inputs ----
            spb_sb = pp.tile([RPC, C], f32, name="spb_sb", tag="spb_sb")
            nc.sync.dma_start(out=spb_sb, in_=spb_d[:, :])
            tpb_sb = pp.tile([RPC, C], f32, name="tpb_sb", tag="tpb_sb")
            nc.sync.dma_start(out=tpb_sb, in_=tpb_d[:, :])
            asc_sb = pp.tile([RPC, 1], f32, name="asc_sb", tag="asc_sb")
            nc.sync.dma_start(out=asc_sb, in_=asc_d[:, :])

            st = {}
            for ch in range(NCHUNK):
                tg = f"c{ch}"
                d = st[ch] = {}
                oh_sb = pp.tile([RPC, LSEG], f32, name=f"oh_sb{ch}", tag=f"oh{tg}")
                nc.sync.dma_start(out=oh_sb, in_=oh_d[ch])
                oht_sb = pp.tile([LSEG, RPC], f32, name=f"oht_sb{ch}", tag=f"oht{tg}")
                nc.sync.dma_start(out=oht_sb, in_=oht_d[ch])
                corrt_sb = pp.tile(
                    [LSEG, 1], f32, name=f"corrt_sb{ch}", tag=f"corrt{tg}"
                )
                nc.sync.dma_start(out=corrt_sb, in_=corrt_d[ch])
                oh_x = pp.tile([RPC, LSEG], xdt, name=f"oh_x{ch}", tag=f"ohx{tg}")
                nc.vector.tensor_copy(oh_x, oh_sb)
                d["oh_sb"], d["oht_sb"], d["corrt_sb"], d["oh_x"] = (
                    oh_sb, oht_sb, corrt_sb, oh_x,
                )

                # ---- load x planes (NDMA class-slices, contiguous) ----
                xts = pp.tile([RPC, C, K], xdt, name=f"xts{ch}", tag=f"xts{tg}")
                for j in range(NDMA):
                    nc.sync.dma_start(
                        out=xts[:, csl[j] : csl[j + 1], :],
                        in_=xs_d[ch, :, csl[j] : csl[j + 1], :],
                    )
                d["xts"] = xts

            # ---- phase 1: exp, class-sum, bit-pattern ln ----
            for ch in range(NCHUNK):
                tg = f"c{ch}"
                d = st[ch]
                et = pp.tile([RPC, C, K], bf16, name=f"et{ch}", tag=f"et{tg}")
                for j in range(NDMA):
                    nc.scalar.activation(
                        et[:, csl[j] : csl[j + 1], :],
                        d["xts"][:, csl[j] : csl[j + 1], :],
                        AF.Exp,
                    )
                d["et"] = et

            for ch in range(NCHUNK):
                tg = f"c{ch}"
                d = st[ch]
                et = d["et"]
                # Pool partial chain over classes [0, CSUM_POOL+1), DVE the rest
                sx = pp.tile([RPC, K], bf16, name=f"sx{ch}", tag=f"sx{tg}")
                nc.gpsimd.tensor_tensor(sx, et[:, 0, :], et[:, 1, :], op=OP.add)
                for c in range(2, CSUM_POOL + 1):
                    nc.gpsimd.tensor_tensor(sx, sx, et[:, c, :], op=OP.add)
                s_t = pp.tile([RPC, K], bf16, name=f"s_t{ch}", tag=f"s_t{tg}")
                nc.vector.tensor_tensor(
                    s_t, et[:, CSUM_POOL + 1, :], et[:, CSUM_POOL + 2, :], op=OP.add
                )
                for c in range(CSUM_POOL + 3, C):
                    nc.vector.tensor_tensor(s_t, s_t, et[:, c, :], op=OP.add)
                nc.vector.tensor_tensor(s_t, s_t, sx, op=OP.add)
                d["s_t"] = s_t

                # lsum[r] = sum_k ln(s) ~ ln2/128 * (sum_k bits(s) - K*B0)
                sbits = s_t.bitcast(u16)
                red = pp.tile([RPC, 1], f32, name=f"red{ch}", tag=f"red{tg}")
                nc.vector.tensor_reduce(red, sbits, axis=AX.X, op=OP.add)
                lsum = pp.tile([RPC, 1], f32, name=f"lsum{ch}", tag=f"lsum{tg}")
                nc.vector.tensor_scalar(
                    lsum, red,
                    scalar1=LN2_128, scalar2=-float(K * B0) * LN2_128,
                    op0=OP.mult, op1=OP.add,
                )
                d["lsum"] = lsum

                # sB[r,k] = bits(s) + B0  (log-domain dividend for phase 2)
                sB = pp.tile([RPC, K], u16, name=f"sB{ch}", tag=f"sB{tg}")
                nc.vector.tensor_scalar(
                    sB, sbits, scalar1=float(B0), scalar2=None, op0=OP.add
                )
                d["sB"] = sB

            # ---- segment table: pacc, B, Bex, w ----
            for ch in range(NCHUNK):
                tg = f"c{ch}"
                d = st[ch]
                pacc = qp.tile([LSEG, C, KB], f32, name=f"pacc{ch}", tag=f"pacc{tg}")
                for kb in range(nkb):
                    nc.tensor.matmul(
                        pacc,
                        d["oh_x"],
                        d["xts"][:, :, kb * KB : (kb + 1) * KB],
                        start=(kb == 0),
                        stop=(kb == nkb - 1),
                    )
                ls_ps = qp.tile([LSEG, 1], f32, name=f"ls_ps{ch}", tag=f"ls_ps{tg}")
                nc.tensor.matmul(ls_ps, d["oh_sb"], d["lsum"], start=True, stop=True)
                d["pacc"], d["ls_ps"] = pacc, ls_ps

            for ch in range(NCHUNK):
                tg = f"c{ch}"
                d = st[ch]
                bxs = pp.tile([LSEG, C], f32, name=f"bxs{ch}", tag=f"bxs{tg}")
                nc.vector.tensor_reduce(bxs, d["pacc"], axis=AX.X, op=OP.add)
                lcol_sb = pp.tile([LSEG, 1], f32, name=f"lcol{ch}", tag=f"lcol{tg}")
                nc.vector.tensor_tensor(
                    lcol_sb, d["ls_ps"], d["corrt_sb"], op=OP.subtract
                )
                bloc_sb = pp.tile([LSEG, C], f32, name=f"bloc{ch}", tag=f"bloc{tg}")
                nc.vector.tensor_scalar(
                    bloc_sb, bxs, scalar1=lcol_sb, scalar2=None, op0=OP.subtract
                )
                # Bex[r, c] = B[seg_of_range[r], c]
                bex_ps = qp.tile([RPC, C], f32, name=f"bex_ps{ch}", tag=f"bex_ps{tg}")
                nc.tensor.matmul(bex_ps, d["oht_sb"], bloc_sb, start=True, stop=True)
                # w = exp(B) * ((lw0-hw0) + (lw1-hw1) * exp(498 B))
                e498_sb = pp.tile([RPC, C], f32, name=f"e498{ch}", tag=f"e498{tg}")
                nc.scalar.activation(e498_sb, bex_ps, AF.Exp, scale=498.0)
                eb1_sb = pp.tile([RPC, C], f32, name=f"eb1{ch}", tag=f"eb1{tg}")
                nc.scalar.activation(eb1_sb, bex_ps, AF.Exp)
                w_sb = pp.tile([RPC, C], f32, name=f"w_sb{ch}", tag=f"w{tg}")
                nc.vector.tensor_tensor(w_sb, tpb_sb, e498_sb, op=OP.mult)
                nc.vector.tensor_tensor(w_sb, w_sb, spb_sb, op=OP.add)
                nc.vector.tensor_tensor(w_sb, w_sb, eb1_sb, op=OP.mult)
                d["w_sb"] = w_sb

            # ---- phase 2: og = (s/et_c) * w_c + a, division in log domain ----
            for ch in range(NCHUNK):
                tg = f"c{ch}"
                d = st[ch]
                et, sB, w_sb = d["et"], d["sB"], d["w_sb"]
                for j in range(NDMA):
                    nsl = csl[j + 1] - csl[j]
                    og = op_pool.tile([RPC, nsl, K], f16, name="og", tag="og", bufs=2)
                    for cl in range(nsl):
                        c = csl[j] + cl
                        g_t = sp.tile([RPC, K], u16, name="g_t", tag="g_t", bufs=3)
                        nc.vector.tensor_tensor(
                            g_t, sB, et[:, c, :].bitcast(u16), op=OP.subtract
                        )
                        if cl < TS_ACT:
                            nc.scalar.activation(
                                og[:, cl, :], g_t.bitcast(bf16), AF.Identity,
                                bias=asc_sb, scale=w_sb[:, c : c + 1],
                            )
                        else:
                            nc.vector.tensor_scalar(
                                og[:, cl, :], g_t.bitcast(bf16),
                                scalar1=w_sb[:, c : c + 1], scalar2=asc_sb,
                                op0=OP.mult, op1=OP.add,
                            )
                    nc.sync.dma_start(
                        out=out_d[ch, :, csl[j] : csl[j + 1], :], in_=og
                    )

    nc.finalize()
    return nc


def _get_nc():
    if "nc" not in _cache:
        _cache["nc"] = _build_nc()
    return _cache["nc"]


def _plan_shards(sp_map):
    """Sort pixels by segment, pack into ranges, bin-pack segments on
    (core, chunk) bins.  Returns per-core dicts with perm (S, -1 = pad) and
    the per-chunk structure tensors."""
    sp = np.asarray(sp_map).ravel()
    order = np.argsort(sp, kind="stable")
    sp_sorted = sp[order]
    starts = np.searchsorted(sp_sorted, np.arange(NSEG), side="left")
    ends = np.searchsorted(sp_sorted, np.arange(NSEG), side="right")
    cnt = ends - starts
    nr = np.where(cnt > 0, -(-cnt // K), 0)

    nbins = NCORES * NCHUNK
    assert int(nr.sum()) <= nbins * RPC, f"range budget exceeded: {nr.sum()}"

    cap = [RPC] * nbins
    nseg_bin = [0] * nbins
    assign = [[] for _ in range(nbins)]
    for s in np.argsort(-nr, kind="stable"):
        s = int(s)
        if nr[s] == 0:
            continue
        best = max(
            (b for b in range(nbins) if cap[b] >= nr[s] and nseg_bin[b] < LSEG - 1),
            key=lambda b: cap[b],
        )
        assign[best].append(s)
        cap[best] -= int(nr[s])
        nseg_bin[best] += 1

    log21 = np.float32(np.log(np.float32(21.0)))
    shards = []
    for kcore in range(NCORES):
        perm = np.full(S, -1, dtype=np.int64)
        ohs, ohts, corrs = [], [], []
        for ch in range(NCHUNK):
            b = kcore * NCHUNK + ch
            seg_of_range = np.full(RPC, LSEG - 1, dtype=np.int64)
            padcnt = np.zeros(LSEG, dtype=np.float64)
            base = ch * S_CH
            r0 = 0
            for ls, s in enumerate(assign[b]):
                n = int(nr[s])
                c0 = int(cnt[s])
                perm[base + r0 * K : base + r0 * K + c0] = order[starts[s] : ends[s]]
                seg_of_range[r0 : r0 + n] = ls
                padcnt[ls] = n * K - c0
                r0 += n
            padcnt[LSEG - 1] = (RPC - r0) * K
            oh = np.zeros((RPC, LSEG), dtype=np.float32)
            oh[np.arange(RPC), seg_of_range] = 1.0
            ohs.append(oh)
            ohts.append(np.ascontiguousarray(oh.T))
            corrs.append((padcnt * float(log21)).astype(np.float32).reshape(LSEG, 1))
        shards.append(
            {
                "perm": perm,
                "oh": np.stack(ohs),
                "oht": np.stack(ohts),
                "corrt": np.stack(corrs),
            }
        )
    return shards


def _prepare_in_maps(inputs):
    q_logits = np.asarray(inputs["q_logits"], dtype=np.float32).reshape(C, NPIX)
    lw = np.asarray(inputs["low_weights"], dtype=np.float32)
    hw = np.asarray(inputs["high_weights"], dtype=np.float32)
    sp_map = np.asarray(inputs["sp_map"])

    shards = _plan_shards(sp_map)

    spn = np.broadcast_to((lw[0] - hw[0]).reshape(1, C), (RPC, C))
    tpn = np.broadcast_to((lw[1] - hw[1]).reshape(1, C), (RPC, C))
    spb = np.ascontiguousarray(spn, dtype=np.float32)
    tpb = np.ascontiguousarray(tpn, dtype=np.float32)
    a_const = np.float32(hw[0]) + np.float32(hw[1])
    asc = np.full((RPC, 1), a_const, dtype=np.float32)

    xdt = _BF16 if IN_DT == "bf16" else _F8
    in_maps = []
    for sh in shards:
        perm = sh["perm"]
        safe = np.where(perm >= 0, perm, 0)
        xs = q_logits[:, safe]
        xs[:, perm < 0] = 0.0
        # [C, S] -> [NCHUNK, RPC, C, K] so device DMAs are contiguous
        xs4 = np.ascontiguousarray(
            xs.reshape(C, NCHUNK, RPC, K).transpose(1, 2, 0, 3).astype(xdt)
        )
        in_maps.append(
            {
                "xs": xs4,
                "oh": sh["oh"],
                "oht": sh["oht"],
                "corrt": sh["corrt"],
                "spb": spb,
                "tpb": tpb,
                "asc": asc,
            }
        )
    return in_maps, shards


def _assemble_output(results, shards):
    out = np.empty((C, NPIX), dtype=np.float32)
    for res, sh in zip(results, shards):
        o4 = np.asarray(res["out"])  # [NCHUNK, RPC, C, K] f16
        o = o4.transpose(2, 0, 1, 3).reshape(C, S).astype(np.float32)
        perm = sh["perm"]
        v = perm >= 0
        out[:, perm[v]] = o[:, v]
    return out.reshape(C, H, W)


def run(inputs, trace=False):
    from concourse.bass_utils import run_bass_kernel_spmd

    nc = _get_nc()
    in_maps, shards = _prepare_in_maps(inputs)
    br = run_bass_kernel_spmd(nc, in_maps, core_ids=list(range(NCORES)), trace=trace)
    out = _assemble_output(br.results, shards)
    return out, br


def kernel(**inputs):
    out, _ = run(inputs, trace=False)
    return out
